# revision 32
# baseline (speedup 1.0000x reference)
"""Trainium2 Bass kernel for the DTI predictor (gnn_message_passing).

Math (reference):
  a_mol = mol_feats @ Wmu[:H] + bmu            [N, heads]
  a_pro = fused_feats @ Wmu[H:]                [P, heads]
  y_atom[n,h] = sum_p ( elu(a_mol[n,h] + a_pro[p,h]) + 1 )
  y = segment_sum(y_atom, mol_batch, B) * 1e-3
  out = elu(y @ W1 + b1) @ W2 + b2             [B, 1]

All-H design. With H = 1{x > 0} (x = am + ap, em = exp(am), ep = exp(ap)):
  elu(x) + 1 = x*H + em*ep*(1-H) + H
  sum_p      = (am+1)*C + A + em*(E - EH)
  where C = sum_p H, A = sum_p ap*H, EH = sum_p ep*H, E = sum_p ep.

Per (head, q-tile of 128 protein atoms) one indicator tile H [128p, 2048n]
is produced in fp8 by either
  DVE: tensor_scalar(bc_am, ap_col, 0, add, is_gt)      (~1.3us, 2x mode)
  ACT: activation(bc_am, Sigmoid, scale=1e4, bias=1e4*ap_col)  (~2.0us)
(saturated sigmoid == step; the decomposition is continuous at x=0 so
boundary rounding is harmless). The PE contracts H-tile PAIRS against a
[ones|ap|ep] fp8 triplet stationary with MatmulPerfMode.DoubleRow (K=256,
2 cols/cycle) accumulating C/A/EH rows [3, 2048] per head in PSUM -- one
cheap PE pass covers BOTH the relu and the min part of 256p x 2048n pairs.
A few pairs run as plain bf16 matmuls on otherwise-idle PE to offload DVE.
Host combine: y from C/A/EH + device am/exp(-am) columns + device fp8
ep sums (esum) for exact consistency, then segment-sum and the tiny MLP.

Sharding: 16 heads across 8 cores (2 each, full N and P replicated).
"""

import sys

sys.path.insert(0, "/opt/trn_rl_repo")

import numpy as np
import ml_dtypes

import concourse.bass as bass
import concourse.tile as tile
import concourse.bacc as bacc
from concourse import mybir
from concourse.bass_utils import run_bass_kernel_spmd

N_MOL, P_PRO, HID, HEADS, B = 2048, 2048, 64, 16, 64
N_CORES = 8
HPC = HEADS // N_CORES          # heads per core = 2
NT = N_MOL // 128               # atom partition-tiles = 16
NQ = P_PRO // 128               # protein q-tiles = 16
NPJ = NQ // 2                   # q-tile pairs = 8
F32 = mybir.dt.float32
BF16 = mybir.dt.bfloat16
F8 = mybir.dt.float8e4
ALU = mybir.AluOpType
AF = mybir.ActivationFunctionType
PM = mybir.MatmulPerfMode

# (h, pj) pairs computed as two plain bf16 matmuls (both H halves on DVE
# at 4x) instead of one fp8 DoubleRow pair -- PE slack absorbs them.
BF_PAIRS = {(0, 0), (1, 0), (0, 4)}
SIG_SCALE = 1e4


def build():
    nc = bacc.Bacc("TRN2", target_bir_lowering=False, debug=False,
                   num_devices=N_CORES)
    molT_d = nc.dram_tensor("molT", [HID + 1, N_MOL], BF16, kind="ExternalInput").ap()
    fusedT_d = nc.dram_tensor("fusedT", [HID, P_PRO], BF16, kind="ExternalInput").ap()
    wmol_d = nc.dram_tensor("wmol", [HID + 1, HPC], BF16, kind="ExternalInput").ap()
    wpro_d = nc.dram_tensor("wpro", [HID, HPC], BF16, kind="ExternalInput").ap()
    wmolr_d = nc.dram_tensor("wmolr", [HID + 1, HPC * 128], BF16, kind="ExternalInput").ap()
    ams_d = nc.dram_tensor("ams", [128, NT * HPC], F32, kind="ExternalOutput").ap()
    rems_d = nc.dram_tensor("rems", [128, NT * HPC], F32, kind="ExternalOutput").ap()
    hrow_d = nc.dram_tensor("hrow", [2 * 3, N_MOL], F32, kind="ExternalOutput").ap()
    esum_d = nc.dram_tensor("esum", [1, 2 * 16], F32, kind="ExternalOutput").ap()

    with tile.TileContext(nc) as tc:
        with (
            tc.tile_pool(name="const", bufs=1) as cpool,
            tc.tile_pool(name="jp", bufs=2) as jppool,
            tc.tile_pool(name="jb", bufs=2) as jbpool,
            tc.tile_pool(name="ps", bufs=1, space=bass.MemorySpace.PSUM) as pspool,
            tc.tile_pool(name="psam", bufs=1, space=bass.MemorySpace.PSUM) as ampool,
            tc.tile_pool(name="psap", bufs=1, space=bass.MemorySpace.PSUM) as appool,
            tc.tile_pool(name="psh", bufs=1, space=bass.MemorySpace.PSUM) as hpool,
        ):
            # ---- inputs ----
            molT = cpool.tile([HID + 1, N_MOL], BF16, tag="molT")
            fusedT = cpool.tile([HID, P_PRO], BF16, tag="fusedT")
            wmol = cpool.tile([HID + 1, HPC], BF16, tag="wmol")
            wpro = cpool.tile([HID, HPC], BF16, tag="wpro")
            wmolr = cpool.tile([HID + 1, HPC * 128], BF16, tag="wmolr")
            nc.gpsimd.dma_start(wmol[:], wmol_d)
            nc.gpsimd.dma_start(wpro[:], wpro_d)
            nc.gpsimd.dma_start(wmolr[:], wmolr_d)
            for j in range(4):
                nc.sync.dma_start(fusedT[:, bass.ts(j, 512)], fusedT_d[:, bass.ts(j, 512)])
            for j in range(4):
                nc.gpsimd.dma_start(molT[:, bass.ts(j, 512)], molT_d[:, bass.ts(j, 512)])

            # ---- a_pro columns (all q) ----
            apc_ps = appool.tile([128, HPC * NQ], F32, tag="apc_ps")
            for q in range(NQ):
                nc.tensor.matmul(apc_ps[:, bass.ts(q, HPC)],
                                 fusedT[:, bass.ts(q, 128)], wpro[:],
                                 start=True, stop=True)
            # scaled bias columns for the ACT sigmoid-step units
            apsig = cpool.tile([128, HPC * NQ], F32, tag="apsig")
            nc.vector.tensor_scalar(apsig[:], apc_ps[:], SIG_SCALE, None,
                                    ALU.mult, ALU.bypass)

            # ---- a_mol columns -> am/rem outputs ----
            am_ps = ampool.tile([128, HPC * NT], F32, tag="am_ps")
            for t in range(NT):
                nc.tensor.matmul(am_ps[:, bass.ts(t, HPC)],
                                 molT[:, bass.ts(t, 128)], wmol[:],
                                 start=True, stop=True)
            am_sb = cpool.tile([128, HPC * NT], F32, tag="am_sb")
            rem_sb = cpool.tile([128, HPC * NT], F32, tag="rem_sb")
            nc.scalar.activation(am_sb[:], am_ps[:], AF.Copy)
            nc.scalar.activation(rem_sb[:], am_ps[:], AF.Exp, scale=-1.0)

            # ---- fp8 DoubleRow triplet stationary ----
            # w3[p, i, 3*s:3*s+3] = [1, ap, ep] for slot s = 2*pj + h, with
            # the pair half i selecting q = 2*pj + i (contiguous triplets).
            epc_sb = cpool.tile([128, HPC * NQ], F8, tag="epc_sb")
            nc.scalar.activation(epc_sb[:], apc_ps[:], AF.Exp)
            # [128, slot, pair-half, 16] so the (h, pj) stationary slice has
            # the exact [[16, 2], [1, 3]] AP form the DoubleRow ISA wants.
            w3 = cpool.tile([128, HPC * NPJ, 2, 16], F8, tag="w3")
            nc.vector.memset(w3[:, :, :, 0:1], 1.0)
            for q in range(NQ):
                pj, i = q // 2, q % 2
                # slots 2*pj (h=0) and 2*pj+1 (h=1) take this q at half i
                nc.vector.tensor_copy(w3[:, 2 * pj:2 * pj + 2, i, 1],
                                      apc_ps[:, bass.ts(q, HPC)])
                nc.vector.tensor_copy(w3[:, 2 * pj:2 * pj + 2, i, 2],
                                      epc_sb[:, bass.ts(q, HPC)])

            # device fp8 ep sums (exact host-side E): upcast the fp8 values
            # to bf16 (lossless) and contract partitions with a bf16 matmul.
            ones16 = cpool.tile([128, 1], BF16, tag="ones16")
            nc.vector.memset(ones16[:], 1.0)
            epc16 = cpool.tile([128, HPC * NQ], BF16, tag="epc16")
            nc.vector.tensor_copy(epc16[:], epc_sb[:])
            esum_ps = ampool.tile([1, HPC * NQ], F32, tag="esum_ps")
            nc.tensor.matmul(esum_ps[:], ones16[:], epc16[:],
                             start=True, stop=True)
            esum_sb = cpool.tile([1, HPC * NQ], F32, tag="esum_sb")
            nc.scalar.activation(esum_sb[:], esum_ps[:], AF.Copy)

            # ---- bc_am broadcast tiles ----
            bc_am = [cpool.tile([128, 4, 512], BF16, tag=f"bcm{h}", name=f"bcm{h}")
                     for h in range(HPC)]
            for h in range(HPC):
                for c in range(4):
                    bc_ps = pspool.tile([128, 512], F32, tag="bc_ps")
                    nc.tensor.matmul(bc_ps[:], wmolr[:, bass.ts(h, 128)],
                                     molT[:, bass.ts(c, 512)], start=True, stop=True)
                    if c % 2 == 0:
                        nc.vector.tensor_copy(bc_am[h][:, c, :], bc_ps[:])
                    else:
                        nc.scalar.activation(bc_am[h][:, c, :], bc_ps[:], AF.Copy)

            # ---- H units + DoubleRow reductions ----
            # DoubleRow matmuls require dst partition 0, so each head gets its
            # own [3, N] PSUM tile (same banks, heads sequential via bufs=1).
            hrow_ps = None

            def h_f8_pair(h, pj, first, last):
                # [128, chunk, pair-half, 512] so each chunk's rhs slice has
                # the exact [[512, 2], [1, 512]] AP form.
                pair = jppool.tile([128, 4, 2, 512], F8, tag="jp")
                q0, q1 = 2 * pj, 2 * pj + 1
                # half i=0 on ACT (saturated sigmoid), i=1 on DVE (is_gt)
                nc.scalar.activation(pair[:, :, 0, :], bc_am[h][:], AF.Sigmoid,
                                     bias=apsig[:, 2 * q0 + h:2 * q0 + h + 1],
                                     scale=SIG_SCALE)
                nc.vector.tensor_scalar(pair[:, :, 1, :], bc_am[h][:],
                                        apc_ps[:, 2 * q1 + h:2 * q1 + h + 1],
                                        0.0, ALU.add, ALU.is_gt)
                for c in range(4):
                    nc.tensor.matmul(hrow_ps[:, bass.ts(c, 512)],
                                     w3[:, 2 * pj + h, :, 0:3],
                                     pair[:, c, :, :],
                                     start=first, stop=last,
                                     perf_mode=PM.DoubleRow)

            def h_bf_pair(h, pj, first, last):
                q0, q1 = 2 * pj, 2 * pj + 1
                for idx, q in enumerate((q0, q1)):
                    tileb = jbpool.tile([128, 4, 512], BF16, tag="jb")
                    nc.vector.tensor_scalar(tileb[:], bc_am[h][:],
                                            apc_ps[:, 2 * q + h:2 * q + h + 1],
                                            0.0, ALU.add, ALU.is_gt)
                    w3b = cpool.tile([128, 3], BF16, tag=f"w3b{h}_{q}",
                                     name=f"w3b{h}_{q}")
                    nc.vector.memset(w3b[:, 0:1], 1.0)
                    nc.vector.tensor_copy(w3b[:, 1:2],
                                          apc_ps[:, 2 * q + h:2 * q + h + 1])
                    nc.scalar.activation(w3b[:, 2:3],
                                         apc_ps[:, 2 * q + h:2 * q + h + 1],
                                         AF.Exp)
                    for c in range(4):
                        nc.tensor.matmul(hrow_ps[:, bass.ts(c, 512)],
                                         w3b[:], tileb[:, c, :],
                                         start=(first and idx == 0),
                                         stop=(last and idx == 1))

            hrow_sb = [cpool.tile([3, N_MOL], F32, tag=f"hsb{h}", name=f"hsb{h}")
                       for h in range(HPC)]
            for h in range(HPC):
                hrow_ps = hpool.tile([3, N_MOL], F32, tag="hrow_ps",
                                     name=f"hrow{h}")
                for pj in range(NPJ):
                    first, last = (pj == 0), (pj == NPJ - 1)
                    if (h, pj) in BF_PAIRS:
                        h_bf_pair(h, pj, first, last)
                    else:
                        h_f8_pair(h, pj, first, last)
                for c in range(2):
                    nc.vector.tensor_copy(hrow_sb[h][:, bass.ts(c, 1024)],
                                          hrow_ps[:, bass.ts(c, 1024)])
                nc.sync.dma_start(hrow_d[3 * h:3 * h + 3, :], hrow_sb[h][:])
            nc.sync.dma_start(ams_d, am_sb[:])
            nc.sync.dma_start(rems_d, rem_sb[:])
            nc.sync.dma_start(esum_d, esum_sb[:])

    nc.compile()
    return nc


_NC = None


def _get_nc():
    global _NC
    if _NC is None:
        _NC = build()
    return _NC


def make_in_maps(mol_feats, fused_feats, Wmu, bmu, mol_batch):
    """Host-side sharding: per-core input dicts."""
    bf = ml_dtypes.bfloat16
    molT = np.concatenate([np.asarray(mol_feats, np.float32).T,
                           np.ones((1, N_MOL), np.float32)], axis=0)
    molT = np.ascontiguousarray(molT).astype(bf)
    fusedT = np.ascontiguousarray(np.asarray(fused_feats, np.float32).T).astype(bf)
    Wmu = np.asarray(Wmu, np.float32)
    bmu = np.asarray(bmu, np.float32)

    in_maps = []
    for c in range(N_CORES):
        h0 = c * HPC
        wmolf = np.concatenate([Wmu[:HID, h0:h0 + HPC], bmu[None, h0:h0 + HPC]],
                               axis=0)
        wmol = np.ascontiguousarray(wmolf).astype(bf)
        wpro = np.ascontiguousarray(Wmu[HID:, h0:h0 + HPC]).astype(bf)
        wmolr = np.ascontiguousarray(np.repeat(wmolf, 128, axis=1)).astype(bf)
        in_maps.append({
            "molT": molT, "fusedT": fusedT,
            "wmol": wmol, "wpro": wpro, "wmolr": wmolr,
        })
    return in_maps


def _elu(v):
    return np.where(v > 0, v, np.expm1(v))


def combine(results, mol_batch, fused_feats=None, Wmu=None):
    """Per-core outputs -> pooled [B, HEADS] f32 (with the *1e-3)."""
    mb = np.asarray(mol_batch).astype(np.int64)
    pooled = np.zeros((B, HEADS), np.float64)
    for c in range(N_CORES):
        ams = results[c]["ams"].astype(np.float64)            # [128, 32]
        rems = results[c]["rems"].astype(np.float64)
        hrow = results[c]["hrow"].astype(np.float64)          # [6, 2048]
        esum = results[c]["esum"].astype(np.float64).reshape(NQ, HPC)
        for h in range(HPC):
            head = c * HPC + h
            am = ams[:, h::HPC].T.reshape(-1)                 # [N] n = t*128+lane
            em = 1.0 / rems[:, h::HPC].T.reshape(-1)
            E = esum[:, h].sum()          # device-fp8 ep summed over all q
            C, A, EH = hrow[3 * h + 0], hrow[3 * h + 1], hrow[3 * h + 2]
            y = (am + 1.0) * C + A + em * (E - EH)
            pooled[:, head] = np.bincount(mb, weights=y, minlength=B)
    return (pooled * 1e-3).astype(np.float32)


def finish(pooled, W1, b1, W2, b2):
    y = _elu(pooled @ np.asarray(W1, np.float32) + np.asarray(b1, np.float32))
    return (y @ np.asarray(W2, np.float32) + np.asarray(b2, np.float32)).astype(np.float32)


def kernel(mol_feats, fused_feats, Wmu, bmu, W1, b1, W2, b2, mol_batch,
           num_graphs, **_unused):
    nc = _get_nc()
    in_maps = make_in_maps(mol_feats, fused_feats, Wmu, bmu, mol_batch)
    res = run_bass_kernel_spmd(nc, in_maps, core_ids=list(range(N_CORES)))
    pooled = combine(res.results, mol_batch)
    return finish(pooled, W1, b1, W2, b2)


# revision 33
# speedup vs baseline: 1.1380x; 1.1380x over previous
"""Trainium2 Bass kernel for the DTI predictor (gnn_message_passing).

Math (reference):
  a_mol = mol_feats @ Wmu[:H] + bmu            [N, heads]
  a_pro = fused_feats @ Wmu[H:]                [P, heads]
  y_atom[n,h] = sum_p ( elu(a_mol[n,h] + a_pro[p,h]) + 1 )
  y = segment_sum(y_atom, mol_batch, B) * 1e-3
  out = elu(y @ W1 + b1) @ W2 + b2             [B, 1]

Identities used (x = am + ap, em = exp(am), ep = exp(ap)):
  elu(x) + 1 = relu(x) + min(em*ep, 1)
  (H-path)   = x*H + em*ep*(1-H) + H           with H = 1{x > 0}
    sum_p over a p-range Q:  (am+1)*C + A + em*(E_Q - EH)
    where C = sum H, A = sum ap*H, EH = sum ep*H, E_Q = sum_Q ep
  (s-paths)  sum_p min(em*ep,1) = sum_p exp(min(x,0)) = em * sum_p min(ep, 1/em)

Engine split per core (2 heads, full N/P replicated; measured HW costs):
  * H-path (p-layout, q-tiles 0..KH-1): DVE tensor_scalar add/is_gt makes the
    0/1 indicator tile at 4x (745ns); PE reduces it against a [ones|ap|ep]
    triplet stationary into per-head [3, N] PSUM rows (C/A/EH). One DVE op +
    one PE pass covers BOTH the relu and the min part of 128 p x 2048 n pairs.
  * n-layout remainder (p in [128*KH, 2048), width W): per (head, atom-tile):
      r: ACT fused relu+bias+accum (exact sum, no correction)
      s: either DVE min(x,0) at 4x + ACT Exp+accum ("S1"), or a single DVE
         1x cache-reduce sum of min(ep, 1/em) ("S2", host multiplies by em).
  Host finishes: y = n-range cols + H combine, segment-sum, tiny MLP.
  (DVE fused dual-op+sum does not exist on HW -- with accum_out, op1 becomes
  the REDUCE op (TensorScalarPtrReduce/CacheReduce, 1x). gpsimd has no float
  ALU ops at all. This split balances DVE/ACT/PE at ~48-50us each.)
"""

import sys

sys.path.insert(0, "/opt/trn_rl_repo")

import numpy as np
import ml_dtypes

import concourse.bass as bass
import concourse.tile as tile
import concourse.bacc as bacc
from concourse import mybir
from concourse.bass_utils import run_bass_kernel_spmd

N_MOL, P_PRO, HID, HEADS, B = 2048, 2048, 64, 16, 64
N_CORES = 8
HPC = HEADS // N_CORES          # heads per core = 2
NT = N_MOL // 128               # atom partition-tiles = 16
F32 = mybir.dt.float32
BF16 = mybir.dt.bfloat16
ALU = mybir.AluOpType
AF = mybir.ActivationFunctionType

KH = 12                         # H-path q-tiles per head (p < 128*KH)
P0 = 128 * KH                   # n-layout p-range start
W = P_PRO - P0                  # n-layout width = 768
S1SET = (0, 4, 8, 12)           # atom-tiles whose s runs DVE-min + ACT-exp
# chunk boundaries for the W-range bc builds (512 then the 256 tail)
WCH = [(0, 512), (512, W)] if W > 512 else ([(0, W)] if W else [])
# cast engine per chunk for bc_am (full width, 4 chunks x 2 heads)
AMCAST = ['D', 'A', 'D', 'A']


def build():
    nc = bacc.Bacc("TRN2", target_bir_lowering=False, debug=False,
                   num_devices=N_CORES)
    molT_d = nc.dram_tensor("molT", [HID + 1, N_MOL], BF16, kind="ExternalInput").ap()
    fusedT_d = nc.dram_tensor("fusedT", [HID, P_PRO], BF16, kind="ExternalInput").ap()
    wmol_d = nc.dram_tensor("wmol", [HID + 1, HPC], BF16, kind="ExternalInput").ap()
    wpro_d = nc.dram_tensor("wpro", [HID, HPC], BF16, kind="ExternalInput").ap()
    wpror_d = nc.dram_tensor("wpror", [HID, HPC * 128], BF16, kind="ExternalInput").ap()
    wmolr_d = nc.dram_tensor("wmolr", [HID + 1, HPC * 128], BF16, kind="ExternalInput").ap()
    acc_d = nc.dram_tensor("acc", [128, NT * HPC * 2], F32, kind="ExternalOutput").ap()
    ams_d = nc.dram_tensor("ams", [128, NT * HPC], F32, kind="ExternalOutput").ap()
    rems_d = nc.dram_tensor("rems", [128, NT * HPC], F32, kind="ExternalOutput").ap()
    hrow_d = nc.dram_tensor("hrow", [2 * 3, N_MOL], F32, kind="ExternalOutput").ap()

    with tile.TileContext(nc) as tc:
        with (
            tc.tile_pool(name="const", bufs=1) as cpool,
            tc.tile_pool(name="jH", bufs=2) as jHpool,
            tc.tile_pool(name="jn", bufs=2) as jnpool,
            tc.tile_pool(name="ju", bufs=2) as jupool,
            tc.tile_pool(name="ja", bufs=2) as japool,
            tc.tile_pool(name="ps", bufs=2, space=bass.MemorySpace.PSUM) as pspool,
            tc.tile_pool(name="psam", bufs=1, space=bass.MemorySpace.PSUM) as ampool,
            tc.tile_pool(name="psap", bufs=1, space=bass.MemorySpace.PSUM) as appool,
            tc.tile_pool(name="psh", bufs=1, space=bass.MemorySpace.PSUM) as hpool,
        ):
            # ---- inputs ----
            molT = cpool.tile([HID + 1, N_MOL], BF16, tag="molT")
            fusedT = cpool.tile([HID, P_PRO], BF16, tag="fusedT")
            wmol = cpool.tile([HID + 1, HPC], BF16, tag="wmol")
            wpro = cpool.tile([HID, HPC], BF16, tag="wpro")
            wpror = cpool.tile([HID, HPC * 128], BF16, tag="wpror")
            wmolr = cpool.tile([HID + 1, HPC * 128], BF16, tag="wmolr")
            nc.gpsimd.dma_start(wmol[:], wmol_d)
            nc.gpsimd.dma_start(wpro[:], wpro_d)
            nc.gpsimd.dma_start(wpror[:], wpror_d)
            nc.gpsimd.dma_start(wmolr[:], wmolr_d)
            for j in range(4):
                nc.sync.dma_start(molT[:, bass.ts(j, 512)], molT_d[:, bass.ts(j, 512)])
            for j in range(4):
                nc.gpsimd.dma_start(fusedT[:, bass.ts(j, 512)], fusedT_d[:, bass.ts(j, 512)])

            # ---- a_mol columns, am/rem ----
            am_ps = ampool.tile([128, HPC * NT], F32, tag="am_ps")
            for t in range(NT):
                nc.tensor.matmul(am_ps[:, bass.ts(t, HPC)],
                                 molT[:, bass.ts(t, 128)], wmol[:],
                                 start=True, stop=True)
            am_sb = cpool.tile([128, HPC * NT], F32, tag="am_sb")
            rem_sb = cpool.tile([128, HPC * NT], F32, tag="rem_sb")
            nc.scalar.activation(am_sb[:], am_ps[:], AF.Copy)
            nc.scalar.activation(rem_sb[:], am_ps[:], AF.Exp, scale=-1.0)

            # ---- a_pro columns (H range) + triplet stationary W3 ----
            apc_ps = appool.tile([128, HPC * KH], F32, tag="apc_ps")
            for q in range(KH):
                nc.tensor.matmul(apc_ps[:, bass.ts(q, HPC)],
                                 fusedT[:, bass.ts(q, 128)], wpro[:],
                                 start=True, stop=True)
            # W3 blocks: [ones x 2KH | ap x 2KH | ep x 2KH]; stationary for
            # (h, q) is the stride-2KH 3-column slice starting at 2q+h.
            nw = HPC * KH
            w3 = cpool.tile([128, 3 * nw], BF16, tag="w3")
            nc.vector.memset(w3[:, 0:nw], 1.0)
            nc.scalar.activation(w3[:, nw:2 * nw], apc_ps[:], AF.Copy)
            nc.scalar.activation(w3[:, 2 * nw:3 * nw], apc_ps[:], AF.Exp)

            # ---- broadcast tiles ----
            bc_am = [cpool.tile([128, N_MOL], BF16, tag=f"bcm{h}", name=f"bcm{h}")
                     for h in range(HPC)]
            bc_ap = [cpool.tile([128, W], BF16, tag=f"bca{h}", name=f"bca{h}")
                     for h in range(HPC)]
            bc_ep = [cpool.tile([128, W], BF16, tag=f"bce{h}", name=f"bce{h}")
                     for h in range(HPC)]
            for h in range(HPC):
                for c in range(4):
                    bc_ps = pspool.tile([128, 512], F32, tag="bc_ps")
                    nc.tensor.matmul(bc_ps[:], wmolr[:, bass.ts(h, 128)],
                                     molT[:, bass.ts(c, 512)], start=True, stop=True)
                    if AMCAST[c] == 'A':
                        nc.scalar.activation(bc_am[h][:, bass.ts(c, 512)],
                                             bc_ps[:], AF.Copy)
                    else:
                        nc.vector.tensor_copy(bc_am[h][:, bass.ts(c, 512)], bc_ps[:])
            for h in range(HPC):
                for (c0, c1) in WCH:
                    cw = c1 - c0
                    bc_ps = pspool.tile([128, 512], F32, tag="bc_ps")
                    nc.tensor.matmul(bc_ps[:, 0:cw], wpror[:, bass.ts(h, 128)],
                                     fusedT[:, P0 + c0:P0 + c1],
                                     start=True, stop=True)
                    if h == 0:
                        nc.vector.tensor_copy(bc_ap[h][:, c0:c1], bc_ps[:, 0:cw])
                    else:
                        nc.scalar.activation(bc_ap[h][:, c0:c1], bc_ps[:, 0:cw],
                                             AF.Copy)
                    nc.scalar.activation(bc_ep[h][:, c0:c1], bc_ps[:, 0:cw], AF.Exp)

            # ---- outputs in SBUF ----
            acc = cpool.tile([128, NT * HPC * 2], F32, tag="acc")
            hrow_ps = hpool.tile([35, N_MOL], F32, tag="hrow_ps")
            HB = (0, 32)                       # PSUM base partition per head

            def h_unit(h, q):
                junk = jHpool.tile([128, N_MOL], BF16, tag="jH")
                nc.vector.tensor_scalar(junk[:], bc_am[h][:],
                                        apc_ps[:, 2 * q + h:2 * q + h + 1], 0.0,
                                        ALU.add, ALU.is_gt)
                for c in range(4):
                    nc.tensor.matmul(hrow_ps[HB[h]:HB[h] + 3, bass.ts(c, 512)],
                                     w3[:, 2 * q + h::nw],
                                     junk[:, bass.ts(c, 512)],
                                     start=(q == 0), stop=(q == KH - 1))

            def r_job(h, t):
                j = 4 * t + 2 * h + 1
                junk = japool.tile([128, W], BF16, tag="ja")
                nc.scalar.activation(junk[:], bc_ap[h][:], AF.Relu,
                                     bias=am_sb[:, 2 * t + h:2 * t + h + 1],
                                     accum_out=acc[:, j:j + 1])

            def s_job(h, t):
                j = 4 * t + 2 * h
                if t in S1SET:
                    u = jupool.tile([128, W], BF16, tag="ju")
                    nc.vector.tensor_scalar(u[:], bc_ap[h][:],
                                            am_sb[:, 2 * t + h:2 * t + h + 1],
                                            0.0, ALU.add, ALU.min)
                    junk = japool.tile([128, W], BF16, tag="ja")
                    nc.scalar.activation(junk[:], u[:], AF.Exp,
                                         accum_out=acc[:, j:j + 1])
                else:
                    junk = jnpool.tile([128, W], BF16, tag="jn")
                    nc.vector.tensor_scalar(junk[:], bc_ep[h][:],
                                            rem_sb[:, 2 * t + h:2 * t + h + 1],
                                            0.0, ALU.min, ALU.add,
                                            accum_out=acc[:, j:j + 1])

            for i in range(NT):
                for h in range(HPC):
                    if i < KH:
                        h_unit(h, i)
                    r_job(h, i)
                    s_job(h, i)

            # ---- drain H rows + outputs ----
            hrow_sb = cpool.tile([35, N_MOL], F32, tag="hrow_sb")
            nc.vector.tensor_copy(hrow_sb[0:3, 0:1024], hrow_ps[0:3, 0:1024])
            nc.vector.tensor_copy(hrow_sb[0:3, 1024:2048], hrow_ps[0:3, 1024:2048])
            nc.vector.tensor_copy(hrow_sb[32:35, 0:1024], hrow_ps[32:35, 0:1024])
            nc.vector.tensor_copy(hrow_sb[32:35, 1024:2048],
                                  hrow_ps[32:35, 1024:2048])
            nc.sync.dma_start(acc_d, acc[:])
            nc.sync.dma_start(ams_d, am_sb[:])
            nc.sync.dma_start(rems_d, rem_sb[:])
            nc.sync.dma_start(hrow_d[0:3, :], hrow_sb[0:3, :])
            nc.sync.dma_start(hrow_d[3:6, :], hrow_sb[32:35, :])

    nc.compile()
    return nc


_NC = None


def _get_nc():
    global _NC
    if _NC is None:
        _NC = build()
    return _NC


def make_in_maps(mol_feats, fused_feats, Wmu, bmu, mol_batch):
    """Host-side sharding: per-core input dicts."""
    bf = ml_dtypes.bfloat16
    molT = np.concatenate([np.asarray(mol_feats, np.float32).T,
                           np.ones((1, N_MOL), np.float32)], axis=0)
    molT = np.ascontiguousarray(molT).astype(bf)
    fusedT = np.ascontiguousarray(np.asarray(fused_feats, np.float32).T).astype(bf)
    Wmu = np.asarray(Wmu, np.float32)
    bmu = np.asarray(bmu, np.float32)

    in_maps = []
    for c in range(N_CORES):
        h0 = c * HPC
        wmolf = np.concatenate([Wmu[:HID, h0:h0 + HPC], bmu[None, h0:h0 + HPC]],
                               axis=0)
        wmol = np.ascontiguousarray(wmolf).astype(bf)
        wpro = np.ascontiguousarray(Wmu[HID:, h0:h0 + HPC]).astype(bf)
        wpror = np.ascontiguousarray(
            np.repeat(Wmu[HID:, h0:h0 + HPC], 128, axis=1)).astype(bf)
        wmolr = np.ascontiguousarray(np.repeat(wmolf, 128, axis=1)).astype(bf)
        in_maps.append({
            "molT": molT, "fusedT": fusedT,
            "wmol": wmol, "wpro": wpro, "wpror": wpror, "wmolr": wmolr,
        })
    return in_maps


def _elu(v):
    return np.where(v > 0, v, np.expm1(v))


def combine(results, mol_batch, fused_feats, Wmu):
    """Per-core outputs -> pooled [B, HEADS] f32 (with the *1e-3)."""
    mb = np.asarray(mol_batch).astype(np.int64)
    ap_host = (np.asarray(fused_feats, np.float64)
               @ np.asarray(Wmu, np.float64)[HID:])          # [P, HEADS]
    E_cov = np.exp(ap_host[:P0]).sum(axis=0)                  # [HEADS]
    pooled = np.zeros((B, HEADS), np.float64)
    for c in range(N_CORES):
        acc = results[c]["acc"].astype(np.float64)            # [128, 64]
        ams = results[c]["ams"].astype(np.float64)            # [128, 32]
        rems = results[c]["rems"].astype(np.float64)          # [128, 32]
        hrow = results[c]["hrow"].astype(np.float64)          # [6, 2048]
        for h in range(HPC):
            head = c * HPC + h
            am = ams[:, h::HPC].T.reshape(-1)                 # [N] n = t*128+lane
            em = 1.0 / rems[:, h::HPC].T.reshape(-1)
            s_cols = acc[:, 2 * h::4]                         # [128, NT]
            r_cols = acc[:, 2 * h + 1::4]
            y = r_cols.T.reshape(-1)                          # ACT relu sums
            s = s_cols.T.reshape(-1).copy()
            s1mask = np.zeros(NT, bool); s1mask[list(S1SET)] = True
            mult = np.where(np.repeat(s1mask, 128), 1.0, em)
            y = y + s * mult
            C, A, EH = hrow[3 * h + 0], hrow[3 * h + 1], hrow[3 * h + 2]
            y = y + (am + 1.0) * C + A + em * (E_cov[head] - EH)
            pooled[:, head] = np.bincount(mb, weights=y, minlength=B)
    return (pooled * 1e-3).astype(np.float32)


def finish(pooled, W1, b1, W2, b2):
    y = _elu(pooled @ np.asarray(W1, np.float32) + np.asarray(b1, np.float32))
    return (y @ np.asarray(W2, np.float32) + np.asarray(b2, np.float32)).astype(np.float32)


def kernel(mol_feats, fused_feats, Wmu, bmu, W1, b1, W2, b2, mol_batch,
           num_graphs, **_unused):
    nc = _get_nc()
    in_maps = make_in_maps(mol_feats, fused_feats, Wmu, bmu, mol_batch)
    res = run_bass_kernel_spmd(nc, in_maps, core_ids=list(range(N_CORES)))
    pooled = combine(res.results, mol_batch, fused_feats, Wmu)
    return finish(pooled, W1, b1, W2, b2)


# revision 36
# speedup vs baseline: 1.1412x; 1.0028x over previous
"""Trainium2 Bass kernel for the DTI predictor (gnn_message_passing).

Math (reference):
  a_mol = mol_feats @ Wmu[:H] + bmu            [N, heads]
  a_pro = fused_feats @ Wmu[H:]                [P, heads]
  y_atom[n,h] = sum_p ( elu(a_mol[n,h] + a_pro[p,h]) + 1 )
  y = segment_sum(y_atom, mol_batch, B) * 1e-3
  out = elu(y @ W1 + b1) @ W2 + b2             [B, 1]

Identities used (x = am + ap, em = exp(am), ep = exp(ap)):
  elu(x) + 1 = relu(x) + min(em*ep, 1)
  (H-path)   = x*H + em*ep*(1-H) + H           with H = 1{x > 0}
    sum_p over a p-range Q:  (am+1)*C + A + em*(E_Q - EH)
    where C = sum H, A = sum ap*H, EH = sum ep*H, E_Q = sum_Q ep
  (s-paths)  sum_p min(em*ep,1) = sum_p exp(min(x,0)) = em * sum_p min(ep, 1/em)

Engine split per core (2 heads, full N/P replicated; measured HW costs):
  * H-path (p-layout, q-tiles 0..KH-1): DVE tensor_scalar add/is_gt makes the
    0/1 indicator tile at 4x (745ns); PE reduces it against a [ones|ap|ep]
    triplet stationary into per-head [3, N] PSUM rows (C/A/EH). One DVE op +
    one PE pass covers BOTH the relu and the min part of 128 p x 2048 n pairs.
  * n-layout remainder (p in [128*KH, 2048), width W): per (head, atom-tile):
      r: ACT fused relu+bias+accum (exact sum, no correction)
      s: either DVE min(x,0) at 4x + ACT Exp+accum ("S1"), or a single DVE
         1x cache-reduce sum of min(ep, 1/em) ("S2", host multiplies by em).
  Host finishes: y = n-range cols + H combine, segment-sum, tiny MLP.
  (DVE fused dual-op+sum does not exist on HW -- with accum_out, op1 becomes
  the REDUCE op (TensorScalarPtrReduce/CacheReduce, 1x). gpsimd has no float
  ALU ops at all. This split balances DVE/ACT/PE at ~48-50us each.)
"""

import sys

sys.path.insert(0, "/opt/trn_rl_repo")

import numpy as np
import ml_dtypes

import concourse.bass as bass
import concourse.tile as tile
import concourse.bacc as bacc
from concourse import mybir
from concourse.bass_utils import run_bass_kernel_spmd

N_MOL, P_PRO, HID, HEADS, B = 2048, 2048, 64, 16, 64
N_CORES = 8
HPC = HEADS // N_CORES          # heads per core = 2
NT = N_MOL // 128               # atom partition-tiles = 16
F32 = mybir.dt.float32
BF16 = mybir.dt.bfloat16
ALU = mybir.AluOpType
AF = mybir.ActivationFunctionType

KH = 13                         # H-path q-tiles per head (p < 128*KH)
P0 = 128 * KH                   # n-layout p-range start
W = P_PRO - P0                  # n-layout width = 768
S1SET = (0, 4, 8, 12)           # atom-tiles whose s runs DVE-min + ACT-exp
# chunk boundaries for the W-range bc builds (512 then the 256 tail)
WCH = [(0, 512), (512, W)] if W > 512 else ([(0, W)] if W else [])
# cast engine per chunk for bc_am (full width, 4 chunks x 2 heads)
AMCAST = ['D', 'A', 'D', 'A']


def build():
    nc = bacc.Bacc("TRN2", target_bir_lowering=False, debug=False,
                   num_devices=N_CORES)
    molT_d = nc.dram_tensor("molT", [HID + 1, N_MOL], BF16, kind="ExternalInput").ap()
    fusedT_d = nc.dram_tensor("fusedT", [HID, P_PRO], BF16, kind="ExternalInput").ap()
    wmol_d = nc.dram_tensor("wmol", [HID + 1, HPC], BF16, kind="ExternalInput").ap()
    wpro_d = nc.dram_tensor("wpro", [HID, HPC], BF16, kind="ExternalInput").ap()
    wpror_d = nc.dram_tensor("wpror", [HID, HPC * 128], BF16, kind="ExternalInput").ap()
    wmolr_d = nc.dram_tensor("wmolr", [HID + 1, HPC * 128], BF16, kind="ExternalInput").ap()
    acc_d = nc.dram_tensor("acc", [128, NT * HPC * 2], F32, kind="ExternalOutput").ap()
    ams_d = nc.dram_tensor("ams", [128, NT * HPC], F32, kind="ExternalOutput").ap()
    rems_d = nc.dram_tensor("rems", [128, NT * HPC], F32, kind="ExternalOutput").ap()
    hrow_d = nc.dram_tensor("hrow", [2 * 3, N_MOL], F32, kind="ExternalOutput").ap()

    with tile.TileContext(nc) as tc:
        with (
            tc.tile_pool(name="const", bufs=1) as cpool,
            tc.tile_pool(name="jH", bufs=2) as jHpool,
            tc.tile_pool(name="jn", bufs=2) as jnpool,
            tc.tile_pool(name="ju", bufs=2) as jupool,
            tc.tile_pool(name="ja", bufs=2) as japool,
            tc.tile_pool(name="ps", bufs=2, space=bass.MemorySpace.PSUM) as pspool,
            tc.tile_pool(name="psam", bufs=1, space=bass.MemorySpace.PSUM) as ampool,
            tc.tile_pool(name="psap", bufs=1, space=bass.MemorySpace.PSUM) as appool,
            tc.tile_pool(name="psh", bufs=1, space=bass.MemorySpace.PSUM) as hpool,
        ):
            # ---- inputs ----
            molT = cpool.tile([HID + 1, N_MOL], BF16, tag="molT")
            fusedT = cpool.tile([HID, P_PRO], BF16, tag="fusedT")
            wmol = cpool.tile([HID + 1, HPC], BF16, tag="wmol")
            wpro = cpool.tile([HID, HPC], BF16, tag="wpro")
            wpror = cpool.tile([HID, HPC * 128], BF16, tag="wpror")
            wmolr = cpool.tile([HID + 1, HPC * 128], BF16, tag="wmolr")
            nc.gpsimd.dma_start(wmol[:], wmol_d)
            nc.gpsimd.dma_start(wpro[:], wpro_d)
            nc.gpsimd.dma_start(wpror[:], wpror_d)
            nc.gpsimd.dma_start(wmolr[:], wmolr_d)
            for j in range(4):
                nc.sync.dma_start(molT[:, bass.ts(j, 512)], molT_d[:, bass.ts(j, 512)])
            for j in range(4):
                nc.gpsimd.dma_start(fusedT[:, bass.ts(j, 512)], fusedT_d[:, bass.ts(j, 512)])

            # ---- a_mol columns, am/rem ----
            am_ps = ampool.tile([128, HPC * NT], F32, tag="am_ps")
            for t in range(NT):
                nc.tensor.matmul(am_ps[:, bass.ts(t, HPC)],
                                 molT[:, bass.ts(t, 128)], wmol[:],
                                 start=True, stop=True)
            am_sb = cpool.tile([128, HPC * NT], F32, tag="am_sb")
            rem_sb = cpool.tile([128, HPC * NT], F32, tag="rem_sb")
            nc.scalar.activation(am_sb[:], am_ps[:], AF.Copy)
            nc.scalar.activation(rem_sb[:], am_ps[:], AF.Exp, scale=-1.0)

            # ---- a_pro columns (H range) + triplet stationary W3 ----
            apc_ps = appool.tile([128, HPC * KH], F32, tag="apc_ps")
            for q in range(KH):
                nc.tensor.matmul(apc_ps[:, bass.ts(q, HPC)],
                                 fusedT[:, bass.ts(q, 128)], wpro[:],
                                 start=True, stop=True)
            # W3 blocks: [ones x 2KH | ap x 2KH | ep x 2KH]; stationary for
            # (h, q) is the stride-2KH 3-column slice starting at 2q+h.
            nw = HPC * KH
            w3 = cpool.tile([128, 3 * nw], BF16, tag="w3")
            nc.vector.memset(w3[:, 0:nw], 1.0)
            nc.scalar.activation(w3[:, nw:2 * nw], apc_ps[:], AF.Copy)
            nc.scalar.activation(w3[:, 2 * nw:3 * nw], apc_ps[:], AF.Exp)

            # ---- broadcast tiles ----
            bc_am = [cpool.tile([128, N_MOL], BF16, tag=f"bcm{h}", name=f"bcm{h}")
                     for h in range(HPC)]
            bc_ap = [cpool.tile([128, W], BF16, tag=f"bca{h}", name=f"bca{h}")
                     for h in range(HPC)]
            bc_ep = [cpool.tile([128, W], BF16, tag=f"bce{h}", name=f"bce{h}")
                     for h in range(HPC)]
            for h in range(HPC):
                for c in range(4):
                    bc_ps = pspool.tile([128, 512], F32, tag="bc_ps")
                    nc.tensor.matmul(bc_ps[:], wmolr[:, bass.ts(h, 128)],
                                     molT[:, bass.ts(c, 512)], start=True, stop=True)
                    if AMCAST[c] == 'A':
                        nc.scalar.activation(bc_am[h][:, bass.ts(c, 512)],
                                             bc_ps[:], AF.Copy)
                    else:
                        nc.vector.tensor_copy(bc_am[h][:, bass.ts(c, 512)], bc_ps[:])
            for h in range(HPC):
                for (c0, c1) in WCH:
                    cw = c1 - c0
                    bc_ps = pspool.tile([128, 512], F32, tag="bc_ps")
                    nc.tensor.matmul(bc_ps[:, 0:cw], wpror[:, bass.ts(h, 128)],
                                     fusedT[:, P0 + c0:P0 + c1],
                                     start=True, stop=True)
                    if h == 0:
                        nc.vector.tensor_copy(bc_ap[h][:, c0:c1], bc_ps[:, 0:cw])
                    else:
                        nc.scalar.activation(bc_ap[h][:, c0:c1], bc_ps[:, 0:cw],
                                             AF.Copy)
                    nc.scalar.activation(bc_ep[h][:, c0:c1], bc_ps[:, 0:cw], AF.Exp)

            # ---- outputs in SBUF ----
            acc = cpool.tile([128, NT * HPC * 2], F32, tag="acc")
            hrow_ps = hpool.tile([35, N_MOL], F32, tag="hrow_ps")
            HB = (0, 32)                       # PSUM base partition per head

            def h_unit(h, q):
                junk = jHpool.tile([128, N_MOL], BF16, tag="jH")
                nc.vector.tensor_scalar(junk[:], bc_am[h][:],
                                        apc_ps[:, 2 * q + h:2 * q + h + 1], 0.0,
                                        ALU.add, ALU.is_gt)
                for c in range(4):
                    nc.tensor.matmul(hrow_ps[HB[h]:HB[h] + 3, bass.ts(c, 512)],
                                     w3[:, 2 * q + h::nw],
                                     junk[:, bass.ts(c, 512)],
                                     start=(q == 0), stop=(q == KH - 1))

            def r_job(h, t):
                j = 4 * t + 2 * h + 1
                junk = japool.tile([128, W], BF16, tag="ja")
                nc.scalar.activation(junk[:], bc_ap[h][:], AF.Relu,
                                     bias=am_sb[:, 2 * t + h:2 * t + h + 1],
                                     accum_out=acc[:, j:j + 1])

            def s_job(h, t):
                j = 4 * t + 2 * h
                if t in S1SET:
                    u = jupool.tile([128, W], BF16, tag="ju")
                    nc.vector.tensor_scalar(u[:], bc_ap[h][:],
                                            am_sb[:, 2 * t + h:2 * t + h + 1],
                                            0.0, ALU.add, ALU.min)
                    junk = japool.tile([128, W], BF16, tag="ja")
                    nc.scalar.activation(junk[:], u[:], AF.Exp,
                                         accum_out=acc[:, j:j + 1])
                else:
                    junk = jnpool.tile([128, W], BF16, tag="jn")
                    nc.vector.tensor_scalar(junk[:], bc_ep[h][:],
                                            rem_sb[:, 2 * t + h:2 * t + h + 1],
                                            0.0, ALU.min, ALU.add,
                                            accum_out=acc[:, j:j + 1])

            for i in range(NT):
                for h in range(HPC):
                    if i < KH:
                        h_unit(h, i)
                    r_job(h, i)
                    s_job(h, i)

            # ---- drain H rows + outputs ----
            hrow_sb = cpool.tile([35, N_MOL], F32, tag="hrow_sb")
            nc.vector.tensor_copy(hrow_sb[0:3, 0:1024], hrow_ps[0:3, 0:1024])
            nc.scalar.activation(hrow_sb[0:3, 1024:2048], hrow_ps[0:3, 1024:2048],
                                 AF.Copy)
            nc.vector.tensor_copy(hrow_sb[32:35, 0:1024], hrow_ps[32:35, 0:1024])
            nc.scalar.activation(hrow_sb[32:35, 1024:2048],
                                 hrow_ps[32:35, 1024:2048], AF.Copy)
            nc.sync.dma_start(acc_d, acc[:])
            nc.sync.dma_start(ams_d, am_sb[:])
            nc.sync.dma_start(rems_d, rem_sb[:])
            nc.sync.dma_start(hrow_d[0:3, :], hrow_sb[0:3, :])
            nc.sync.dma_start(hrow_d[3:6, :], hrow_sb[32:35, :])

    nc.compile()
    return nc


_NC = None


def _get_nc():
    global _NC
    if _NC is None:
        _NC = build()
    return _NC


def make_in_maps(mol_feats, fused_feats, Wmu, bmu, mol_batch):
    """Host-side sharding: per-core input dicts."""
    bf = ml_dtypes.bfloat16
    molT = np.concatenate([np.asarray(mol_feats, np.float32).T,
                           np.ones((1, N_MOL), np.float32)], axis=0)
    molT = np.ascontiguousarray(molT).astype(bf)
    fusedT = np.ascontiguousarray(np.asarray(fused_feats, np.float32).T).astype(bf)
    Wmu = np.asarray(Wmu, np.float32)
    bmu = np.asarray(bmu, np.float32)

    in_maps = []
    for c in range(N_CORES):
        h0 = c * HPC
        wmolf = np.concatenate([Wmu[:HID, h0:h0 + HPC], bmu[None, h0:h0 + HPC]],
                               axis=0)
        wmol = np.ascontiguousarray(wmolf).astype(bf)
        wpro = np.ascontiguousarray(Wmu[HID:, h0:h0 + HPC]).astype(bf)
        wpror = np.ascontiguousarray(
            np.repeat(Wmu[HID:, h0:h0 + HPC], 128, axis=1)).astype(bf)
        wmolr = np.ascontiguousarray(np.repeat(wmolf, 128, axis=1)).astype(bf)
        in_maps.append({
            "molT": molT, "fusedT": fusedT,
            "wmol": wmol, "wpro": wpro, "wpror": wpror, "wmolr": wmolr,
        })
    return in_maps


def _elu(v):
    return np.where(v > 0, v, np.expm1(v))


def combine(results, mol_batch, fused_feats, Wmu):
    """Per-core outputs -> pooled [B, HEADS] f32 (with the *1e-3)."""
    mb = np.asarray(mol_batch).astype(np.int64)
    ap_host = (np.asarray(fused_feats, np.float64)
               @ np.asarray(Wmu, np.float64)[HID:])          # [P, HEADS]
    E_cov = np.exp(ap_host[:P0]).sum(axis=0)                  # [HEADS]
    pooled = np.zeros((B, HEADS), np.float64)
    for c in range(N_CORES):
        acc = results[c]["acc"].astype(np.float64)            # [128, 64]
        ams = results[c]["ams"].astype(np.float64)            # [128, 32]
        rems = results[c]["rems"].astype(np.float64)          # [128, 32]
        hrow = results[c]["hrow"].astype(np.float64)          # [6, 2048]
        for h in range(HPC):
            head = c * HPC + h
            am = ams[:, h::HPC].T.reshape(-1)                 # [N] n = t*128+lane
            em = 1.0 / rems[:, h::HPC].T.reshape(-1)
            s_cols = acc[:, 2 * h::4]                         # [128, NT]
            r_cols = acc[:, 2 * h + 1::4]
            y = r_cols.T.reshape(-1)                          # ACT relu sums
            s = s_cols.T.reshape(-1).copy()
            s1mask = np.zeros(NT, bool); s1mask[list(S1SET)] = True
            mult = np.where(np.repeat(s1mask, 128), 1.0, em)
            y = y + s * mult
            C, A, EH = hrow[3 * h + 0], hrow[3 * h + 1], hrow[3 * h + 2]
            y = y + (am + 1.0) * C + A + em * (E_cov[head] - EH)
            pooled[:, head] = np.bincount(mb, weights=y, minlength=B)
    return (pooled * 1e-3).astype(np.float32)


def finish(pooled, W1, b1, W2, b2):
    y = _elu(pooled @ np.asarray(W1, np.float32) + np.asarray(b1, np.float32))
    return (y @ np.asarray(W2, np.float32) + np.asarray(b2, np.float32)).astype(np.float32)


def kernel(mol_feats, fused_feats, Wmu, bmu, W1, b1, W2, b2, mol_batch,
           num_graphs, **_unused):
    nc = _get_nc()
    in_maps = make_in_maps(mol_feats, fused_feats, Wmu, bmu, mol_batch)
    res = run_bass_kernel_spmd(nc, in_maps, core_ids=list(range(N_CORES)))
    pooled = combine(res.results, mol_batch, fused_feats, Wmu)
    return finish(pooled, W1, b1, W2, b2)
